# revision 17
# baseline (speedup 1.0000x reference)
"""Causal self-attention (B=4, T=2048, D=1024, H=16) on 8 trn2 NeuronCores.

Sharding: core c handles batch b=c//2 and head-group hg=c%2 (8 of 16 heads).
W_q/W_k/W_v are column-sharded per head-group (host-side). After attention,
each pair of cores AllGathers the transposed attention output (channels) and
computes a disjoint half of the output channels of the O-projection
(W_o.T column-sharded per rank parity), so the device program is identical
on every core; the host concatenates the halves.

All matmuls run in bf16 with fp32 PSUM accumulation. Softmax is computed
without max-subtraction (scores are O(1) here; exp is safe in fp32/bf16),
with the denominator obtained from an extra ones-column appended to V.
The AV product is computed V-stationary as out^T[c, q] = V^T A with wide
free-dim matmuls, which lands the attention output directly in the
transposed layout the O-projection needs (no PE transposes); the softmax
denominator arrives as psum row 64 and is divided out via a gpsimd
partition_broadcast + DVE multiply.
"""

import os
import sys

for _p in ("/opt/trn_rl_repo", "/root/.axon_site/_ro/trn_rl_repo"):
    if os.path.isdir(_p) and _p not in sys.path:
        sys.path.insert(0, _p)

import ml_dtypes
import numpy as np

import concourse.bass as bass  # noqa: F401  (AP helpers)
import concourse.mybir as mybir
import concourse.tile as tile
from concourse.bacc import Bacc
from concourse.bass_utils import run_bass_kernel_spmd
from concourse import library_config

B = 4
D = 1024
H = 16
DH = 64
N_CORES = 8
HG = 2              # tensor-parallel degree within a batch (head groups)
CL = D // HG        # 512 local channels (8 heads) per core
SCALE = 1.0 / 8.0   # 1 / sqrt(DH)

BF16 = mybir.dt.bfloat16
F32 = mybir.dt.float32
NPBF16 = ml_dtypes.bfloat16
EXP = mybir.ActivationFunctionType.Exp

# Default sequence length; build_nc(T) is parametric for testing.
T_FULL = 2048


def build_nc(T):
    NT = T // 128          # t-tiles
    ND = D // 128          # d-tiles (8)
    NCT = CL // 128        # local c-tiles / head pairs (4)
    NJ = T // 512          # tq chunks
    assert T % 512 == 0

    nc = Bacc(None)
    xT = nc.dram_tensor("xT", [D, T], BF16, kind="ExternalInput")
    wqT = nc.dram_tensor("wqT", [D, CL], BF16, kind="ExternalInput")
    wkT = nc.dram_tensor("wkT", [D, CL], BF16, kind="ExternalInput")
    wvT = nc.dram_tensor("wvT", [D, CL], BF16, kind="ExternalInput")
    woT = nc.dram_tensor("woT", [D, CL], BF16, kind="ExternalInput")
    mask = nc.dram_tensor("mask", [128, 128], BF16, kind="ExternalInput")
    y = nc.dram_tensor("y", [T, CL], F32, kind="ExternalOutput")

    with tile.TileContext(nc) as tc:
        with (
            tc.tile_pool(name="const", bufs=1) as constp,
            tc.tile_pool(name="wo", bufs=1) as wop,
            tc.tile_pool(name="qk", bufs=1) as qkp,
            tc.tile_pool(name="vaug", bufs=1) as vaugp,
            tc.tile_pool(name="outT", bufs=1) as outTp,
            tc.tile_pool(name="ag0", bufs=1) as ag0p,
            tc.tile_pool(name="dram", bufs=1, space="DRAM") as dramp,
        ):
            # gpsimd ucode library with partition_broadcast (softmax denom
            # broadcast); loaded first so every later gpsimd op sees it.
            nc.gpsimd.load_library(library_config.attn)

            mask_sb = constp.tile([128, 128], BF16, tag="mask", name="maskt")
            nc.sync.dma_start(mask_sb[:], mask[:])

            # wo tiles created here; DMA deferred until after x (wo is only
            # needed by the output projection at the end of the kernel).
            wo_sb = []
            for ct in range(ND):
                t = wop.tile([128, CL], BF16, tag=f"wo{ct}", name=f"wo{ct}")
                wo_sb.append(t)

            qt_sb = [qkp.tile([128, T], BF16, tag=f"q{ct}", name=f"q{ct}") for ct in range(NCT)]
            kt_sb = [qkp.tile([128, T], BF16, tag=f"k{ct}", name=f"k{ct}") for ct in range(NCT)]
            vaug_sb = [vaugp.tile([128, 8 * 65], BF16, tag=f"v{tt}", name=f"v{tt}") for tt in range(NT)]
            outT_sb = [outTp.tile([128, T], BF16, tag=f"o{ct}", name=f"o{ct}") for ct in range(NCT)]

            TH = T // 2
            ag_in = [[dramp.tile([128, TH], BF16, tag=f"agi{hp}_{hf}", name=f"agi{hp}_{hf}")
                      for hf in range(2)] for hp in range(NCT)]
            ag_out = [[dramp.tile([256, TH], BF16, tag=f"ago{hp}_{hf}", name=f"ago{hp}_{hf}")
                       for hf in range(2)] for hp in range(NCT)]

            with (
                tc.tile_pool(name="xtw", bufs=1) as xtwp,
                tc.tile_pool(name="qkvps", bufs=1, space="PSUM") as qkvpsp,
            ):
                # PE warmup: keep the systolic array active through the
                # initial DMA window so HAM reaches (and keeps) K=8/8.
                junk = xtwp.tile([128, 512], BF16, tag="junk", name="junk")
                nc.vector.memset(junk[:], 0.5)

                # Weights go out on the gpsimd DMA ring, x on the sync ring —
                # the two transfer streams overlap. x is split into two
                # column-halves so attention can begin after half A lands.
                xt_sb, wq_sb, wk_sb, wv_sb = [], [], [], []
                for wname, w_dram, lst in (("wq", wqT, wq_sb), ("wk", wkT, wk_sb),
                                           ("wv", wvT, wv_sb)):
                    for dt in range(ND):
                        t = xtwp.tile([128, CL], BF16, tag=f"{wname}{dt}", name=f"{wname}{dt}")
                        nc.gpsimd.dma_start(t[:], w_dram[dt * 128:(dt + 1) * 128, :])
                        lst.append(t)
                for dt in range(ND):
                    t = xtwp.tile([128, T], BF16, tag=f"x{dt}", name=f"x{dt}")
                    xt_sb.append(t)
                TH2 = T // 2
                for half in range(2):
                    cs = slice(half * TH2, (half + 1) * TH2)
                    for dt in range(ND):
                        nc.sync.dma_start(xt_sb[dt][:, cs], xT[dt * 128:(dt + 1) * 128, cs])
                for ct in range(ND):
                    nc.gpsimd.dma_start(wo_sb[ct][:], woT[ct * 128:(ct + 1) * 128, :])

                with tc.tile_pool(name="warmps", bufs=1, space="PSUM") as warmpsp:
                    wps = warmpsp.tile([128, 512], F32, tag="wps", name="wps")
                    for _ in range(20):
                        nc.tensor.matmul(wps[:], junk[:, 0:128], junk[:],
                                         start=True, stop=True)

                # ---- QKV emit helpers (upfront + attention fillers) ----
                def emit_qt(w_sb, dst, ct, tq, pool=None):
                    ps = (pool or qkvpsp).tile([128, 512], F32, tag="qkvps", name="qkvps")
                    for dt in range(ND):
                        nc.tensor.matmul(
                            ps[:],
                            w_sb[dt][:, ct * 128:(ct + 1) * 128],
                            xt_sb[dt][:, tq * 512:(tq + 1) * 512],
                            start=(dt == 0), stop=(dt == ND - 1),
                        )
                    nc.vector.tensor_copy(dst[ct][:, tq * 512:(tq + 1) * 512], ps[:])

                def emit_v(tt, pool=None):
                    ps = (pool or qkvpsp).tile([128, 512], F32, tag="qkvps", name="qkvps")
                    for dt in range(ND):
                        nc.tensor.matmul(
                            ps[:],
                            xt_sb[dt][:, tt * 128:(tt + 1) * 128],
                            wv_sb[dt][:],
                            start=(dt == 0), stop=(dt == ND - 1),
                        )
                    nc.vector.memset(vaug_sb[tt][:], 1.0)
                    dst = vaug_sb[tt][:].rearrange("p (h e) -> p h e", e=65)[:, :, 0:64]
                    src = ps[:].rearrange("p (h e) -> p h e", e=64)
                    nc.vector.tensor_copy(dst, src)

                # upfront: only what iteration (hp=0, J=0) needs. Run the six
                # emit chains on six distinct PSUM banks so they all progress
                # concurrently as the x tiles land from DMA (a single shared
                # bank would serialize them behind the full x transfer).
                with tc.tile_pool(name="upps", bufs=5, space="PSUM") as uppsp:
                    emit_qt(wq_sb, qt_sb, 0, 0, pool=uppsp)
                    emit_qt(wk_sb, kt_sb, 0, 0, pool=uppsp)
                    for tt in range(3):
                        emit_v(tt, pool=uppsp)
                    emit_v(3)

                # deferred QKV work, tagged with the work-iteration index
                # (hp*NJ+J) that first consumes it
                fillers = []  # (deadline_idx, closure)
                for tt in range(4, NT):
                    fillers.append((tt // 4, lambda tt=tt: emit_v(tt)))
                for ct in range(1, NCT):
                    for c in range(NJ):
                        fillers.append(
                            (ct * NJ + c, lambda ct=ct, c=c: emit_qt(wq_sb, qt_sb, ct, c)))
                        fillers.append(
                            (ct * NJ + c, lambda ct=ct, c=c: emit_qt(wk_sb, kt_sb, ct, c)))
                for c in range(1, NJ):
                    fillers.append((c, lambda c=c: emit_qt(wq_sb, qt_sb, 0, c)))
                    fillers.append((c, lambda c=c: emit_qt(wk_sb, kt_sb, 0, c)))
                fillers.sort(key=lambda x: x[0])

                # ---------------- Attention ----------------
                with (
                    tc.tile_pool(name="att", bufs=26) as attp,
                    tc.tile_pool(name="rr", bufs=2) as rrp,
                    tc.tile_pool(name="rb", bufs=2) as rbp,
                    tc.tile_pool(name="stps", bufs=2, space="PSUM") as stpsp,
                    tc.tile_pool(name="avps", bufs=1, space="PSUM") as avpsp,
                ):
                    def emit_qk_tile(hp, J, i, atts):
                        st = stpsp.tile([128, 1024], F32, tag="st", name="st")
                        for h in range(2):
                            nc.tensor.matmul(
                                st[:, h * 512:(h + 1) * 512],
                                kt_sb[hp][h * 64:(h + 1) * 64, i * 128:(i + 1) * 128],
                                qt_sb[hp][h * 64:(h + 1) * 64, J * 512:(J + 1) * 512],
                                start=True, stop=True, tile_position=(h * 64, 0),
                            )
                        att = attp.tile([128, 1024], BF16, tag="att", name="att")
                        k = i - 4 * J
                        if k <= 0:
                            nc.scalar.activation(att[:, 0:1024], st[:, 0:1024], EXP, scale=SCALE)
                        else:
                            # diagonal region: exp only the consumed suffix of
                            # each head's 512-col block (3D AP skips the rest)
                            o = k * 128
                            st3 = st[:].rearrange("p (h q) -> p h q", h=2)[:, :, o:512]
                            at3 = att[:].rearrange("p (h q) -> p h q", h=2)[:, :, o:512]
                            nc.scalar.activation(at3, st3, EXP, scale=SCALE)
                        if k >= 0:  # diagonal 128-block: keep tk <= tq
                            for h in range(2):
                                lo = h * 512 + k * 128
                                nc.vector.tensor_mul(
                                    att[:, lo:lo + 128], att[:, lo:lo + 128], mask_sb[:]
                                )
                        atts.append(att)

                    def emit_norm(hp, J, h, av):
                        # av: [65, 512] psum — rows 0-63 = V^T A (channels),
                        # row 64 = ones^T A = softmax denominator per q.
                        rr = rrp.tile([1, 512], F32, tag="rr", name="rr")
                        nc.vector.reciprocal(rr[:], av[64:65, :])
                        rb = rbp.tile([64, 512], F32, tag="rb", name="rb")
                        nc.gpsimd.partition_broadcast(rb[:], rr[:], channels=64)
                        nc.vector.tensor_mul(
                            outT_sb[hp][h * 64:(h + 1) * 64, J * 512:(J + 1) * 512],
                            av[0:64, :],
                            rb[:],
                        )

                    def emit_ag(hp, hf):
                        nc.gpsimd.dma_start(
                            ag_in[hp][hf][:], outT_sb[hp][:, hf * TH:(hf + 1) * TH])
                        nc.gpsimd.collective_compute(
                            "AllGather",
                            mybir.AluOpType.bypass,
                            replica_groups=[[0, 1], [2, 3], [4, 5], [6, 7]],
                            ins=[ag_in[hp][hf].opt()],
                            outs=[ag_out[hp][hf].opt()],
                        )

                    def make_av_items(hp, J, atts):
                        # out^T[c, q] = V^T A via V-stationary wide-F matmuls:
                        # one [65, <=512] matmul per (k-tile, head), accumulated
                        # over k-tiles in PSUM. Row 64 (ones col of vaug) is
                        # the softmax denominator.
                        n_tk = 4 * J + 4
                        avh = [avpsp.tile([65, 512], F32, tag=f"avh{h}", name=f"avh{h}")
                               for h in range(2)]
                        items = []
                        for i in range(n_tk):
                            o = max(0, i - 4 * J) * 128
                            for h in range(2):
                                hl = hp * 2 + h
                                items.append(
                                    lambda i=i, h=h, o=o, hl=hl, av=avh[h],
                                           atts=atts, n_tk=n_tk:
                                    nc.tensor.matmul(
                                        av[:, o:512],
                                        vaug_sb[i][:, hl * 65:(hl + 1) * 65],
                                        atts[i][:, h * 512 + o:(h + 1) * 512],
                                        start=(i == 0), stop=(i == n_tk - 1),
                                    )
                                )
                        for h in range(2):
                            items.append(
                                lambda hp=hp, J=J, h=h, av=avh[h]:
                                emit_norm(hp, J, h, av)
                            )
                        return items

                    # hf=0 halves of the AllGather output, prefetched into
                    # SBUF as soon as each head-pair's first AG lands so the
                    # first half of the output projection starts DMA-free.
                    ag0_sb = [None] * ND

                    def prefetch_ag0(hp):
                        for ct in (hp, hp + NCT):
                            t = ag0p.tile([128, TH], BF16, tag=f"ag0_{ct}", name=f"ag0_{ct}")
                            rows = slice(0, 128) if ct < NCT else slice(128, 256)
                            nc.sync.dma_start(t[:], ag_out[hp][0][rows, :])
                            ag0_sb[ct] = t

                    work = [(hp, J) for hp in range(NCT) for J in range(NJ)]
                    av_queue = []
                    fpos = 0
                    for idx, (hp, J) in enumerate(work):
                        if J == NJ - 1:  # AG(hp, hf=0) issued last iteration
                            prefetch_ag0(hp)
                        n_tk = 4 * J + 4
                        atts = []
                        # interleave: previous iteration's AV work + QKV
                        # fillers due before the NEXT iteration starts
                        due = []
                        lookahead = 3 if idx < 4 else 1
                        while fpos < len(fillers) and fillers[fpos][0] <= idx + lookahead:
                            due.append(fillers[fpos][1])
                            fpos += 1
                        mixed = []
                        na, nd = len(av_queue), len(due)
                        ai = di = 0
                        for s in range(na + nd):
                            if ai * nd <= di * na and ai < na:
                                mixed.append(av_queue[ai]); ai += 1
                            elif di < nd:
                                mixed.append(due[di]); di += 1
                            else:
                                mixed.append(av_queue[ai]); ai += 1
                        total = len(mixed)
                        done = 0
                        for i in range(n_tk):
                            emit_qk_tile(hp, J, i, atts)
                            want = ((i + 1) * total) // n_tk
                            while done < want:
                                mixed[done]()
                                done += 1
                        while done < total:
                            mixed[done]()
                            done += 1
                        av_queue = make_av_items(hp, J, atts)
                        if 2 * (J + 1) % NJ == 0:  # after J == NJ//2-1 and NJ-1
                            hf = (2 * (J + 1)) // NJ - 1
                            av_queue.append(lambda hp=hp, hf=hf: emit_ag(hp, hf))
                    for c in av_queue:
                        c()

            # ---------------- Output projection ----------------
            # hf=0 halves (tt 0-7) were prefetched during attention; load the
            # hf=1 halves now (fires as soon as each AG lands) and overlap
            # with the hf=0 matmul groups.
            with (
                tc.tile_pool(name="ag1", bufs=1) as ag1p,
                tc.tile_pool(name="ysb", bufs=3) as ysbp,
                tc.tile_pool(name="yps", bufs=8, space="PSUM") as ypsp,
            ):
                ag1_sb = [None] * ND
                for ct in [0, 1, 4, 5, 2, 6, 3, 7]:
                    t = ag1p.tile([128, TH], BF16, tag=f"ag1_{ct}", name=f"ag1_{ct}")
                    hp = ct if ct < NCT else ct - NCT
                    rows = slice(0, 128) if ct < NCT else slice(128, 256)
                    eng = nc.gpsimd if ct in (3, 7) else nc.sync
                    eng.dma_start(t[:], ag_out[hp][1][rows, :])
                    ag1_sb[ct] = t
                n_grp = NT // 4
                ct_order = [0, 1, 4, 5, 2, 6, 3, 7]  # last-AG-dependent last
                for grp in range(4):
                    ag_half = ag0_sb if grp < 2 else ag1_sb
                    yps = [ypsp.tile([128, 512], F32, tag="yps", name="yps") for _ in range(n_grp)]
                    for cti, ct in enumerate(ct_order):
                        for tti in range(n_grp):
                            tt = grp * n_grp + tti
                            th = tt % (NT // 2)
                            nc.tensor.matmul(
                                yps[tti][:],
                                ag_half[ct][:, th * 128:(th + 1) * 128],
                                wo_sb[ct][:],
                                start=(cti == 0), stop=(cti == ND - 1),
                            )
                    for tti in range(n_grp):
                        tt = grp * n_grp + tti
                        ysb = ysbp.tile([128, 512], F32, tag="ysb", name="ysb")
                        # split psum->sbuf copies across DVE and ScalarE (both
                        # idle here) so the drain doesn't serialize on one
                        if tti % 2 == 0:
                            nc.vector.tensor_copy(ysb[:], yps[tti][:])
                        else:
                            nc.scalar.copy(ysb[:], yps[tti][:])
                        nc.sync.dma_start(y[tt * 128:(tt + 1) * 128, :], ysb[:])

    nc.compile()
    return nc


_NC_CACHE = {}


def _get_nc(T):
    if T not in _NC_CACHE:
        _NC_CACHE[T] = build_nc(T)
    return _NC_CACHE[T]


def shard_inputs(x, W_q, W_k, W_v, W_o):
    """Host-side sharding: per-core input dicts (bf16, transposed)."""
    T = x.shape[1]
    tri = np.triu(np.ones((128, 128), np.float32)).astype(NPBF16)
    in_maps = []
    for c in range(N_CORES):
        b, hg = c // 2, c % 2
        cs = slice(hg * CL, (hg + 1) * CL)
        in_maps.append({
            "xT": np.ascontiguousarray(x[b].T).astype(NPBF16),
            "wqT": np.ascontiguousarray(W_q[cs, :].T).astype(NPBF16),
            "wkT": np.ascontiguousarray(W_k[cs, :].T).astype(NPBF16),
            "wvT": np.ascontiguousarray(W_v[cs, :].T).astype(NPBF16),
            "woT": np.ascontiguousarray(W_o[cs, :].T).astype(NPBF16),
            "mask": tri,
        })
    return in_maps


def assemble_output(results, T):
    y = np.zeros((B, T, D), np.float32)
    for c in range(N_CORES):
        b, hg = c // 2, c % 2
        y[b][:, hg * CL:(hg + 1) * CL] = results[c]["y"]
    return y


def kernel(x, W_q, W_k, W_v, W_o, _trace=False):
    x = np.asarray(x, dtype=np.float32)
    W_q = np.asarray(W_q, dtype=np.float32)
    W_k = np.asarray(W_k, dtype=np.float32)
    W_v = np.asarray(W_v, dtype=np.float32)
    W_o = np.asarray(W_o, dtype=np.float32)
    T = x.shape[1]
    nc = _get_nc(T)
    in_maps = shard_inputs(x, W_q, W_k, W_v, W_o)
    res = run_bass_kernel_spmd(
        nc, in_maps, core_ids=list(range(N_CORES)), trace=_trace
    )
    out = assemble_output(res.results, T)
    if _trace:
        return out, res
    return out

